# revision 17
# baseline (speedup 1.0000x reference)
"""Trainium2 Bass kernel for nn_GAT_Mixer (8-layer GNN: GCN x2, TransformerConv x2, GAT x4).

Strategy (8 NeuronCores, node-sharded, dense-adjacency formulation):
  - Nodes row-sharded: core c owns rows [512c, 512(c+1)).
  - All segment ops become dense matmuls against a [4096 src, 512 dst_local]
    column slice of the (transposed) adjacency/attention matrix per core.
  - GCN: sym-normalized adjacency precomputed dense on host (bf16), streamed.
  - GAT/Transformer: per-edge softmax == masked dense softmax; mask = edge
    multiplicity matrix (host, bf16, streamed per 128-src-row tile).
    araw[s,d] = mult[s,d]*exp(logit[s,d]); out = (araw^T @ h)/rowsum.
  - Aggregation is kt-outer streamed: for each 128-src tile, build the araw
    tile (DVE/ACT) and immediately accumulate into 4 persistent PSUM banks
    (+1 rowsum rider bank) so TensorE stays busy and HAM-warm.
  - All aggregations contract at feature dim 512 via A@(xW) = (A@x)W
    reordering; GAT attention vectors folded a' = W@a on host; tr4 logits use
    the bilinear trick q.k = x (Wk Wq^T) x^T at dim 512.
  - Cross-core: AllGather of 512-row local activations (bf16) per layer.

kernel(**inputs) takes FULL inputs, returns (h2, h4, h6, h8) each
[1, 4096, 1024] float32, matching reference().
"""

import os
import sys

sys.path.insert(0, "/opt/trn_rl_repo")
if "/root/.axon_site" not in sys.path:
    sys.path.insert(0, "/root/.axon_site")

import numpy as np
import ml_dtypes
from contextlib import ExitStack

import concourse.bass as bass
import concourse.mybir as mybir
import concourse.tile as tile
from concourse import bacc
from concourse import bass_utils
from concourse import masks as cmasks
from concourse.bass_interp import get_hw_module

BF16 = ml_dtypes.bfloat16
FP32 = mybir.dt.float32
BF = mybir.dt.bfloat16
AF = mybir.ActivationFunctionType
OP = mybir.AluOpType

NCORES = 8
N = 4096
NL = N // NCORES  # 512 local nodes per core
D1 = 1024
D2 = 512
TN = N // 128  # 32 global node tiles
TL = NL // 128  # 4 local node tiles
C = 512  # canonical free-dim width
RG = [list(range(NCORES))]

_CACHE = {}


def _bf(a):
    return np.ascontiguousarray(np.asarray(a, dtype=np.float32).astype(BF16))


def _f32(a):
    return np.ascontiguousarray(np.asarray(a), dtype=np.float32)


# ----------------------------------------------------------------------------
# Host-side preprocessing
# ----------------------------------------------------------------------------


def _host_prep(x, edge_index, params):
    x = _f32(x)
    ei = np.asarray(edge_index)
    src = ei[0].astype(np.int64)
    dst = ei[1].astype(np.int64)
    loop = np.arange(N, dtype=np.int64)
    src_sl = np.concatenate([src, loop])
    dst_sl = np.concatenate([dst, loop])

    deg = np.zeros(N, dtype=np.float32)
    np.add.at(deg, dst_sl, 1.0)
    dinv = np.where(deg > 0, np.maximum(deg, 1.0) ** -0.5, 0.0).astype(np.float32)
    A = np.zeros((N, N), dtype=np.float32)
    np.add.at(A, (dst_sl, src_sl), dinv[src_sl] * dinv[dst_sl])

    Msl = np.zeros((N, N), dtype=np.float32)
    np.add.at(Msl, (dst_sl, src_sl), 1.0)
    Mnosl = np.zeros((N, N), dtype=np.float32)
    np.add.at(Mnosl, (dst, src), 1.0)

    p = params

    def W(q):
        return _f32(q["W"])

    def b(q):
        return _f32(q["b"])

    for name in ("gcn1", "gcn2", "gat5", "gat6", "gat7", "gat8"):
        assert not np.any(b(p[name])), f"nonzero bias {name} unsupported"
    for name in ("tr3", "tr4"):
        for sub in ("q", "k", "v", "skip"):
            assert not np.any(b(p[name][sub])), f"nonzero bias {name}.{sub}"

    w5, w6 = W(p["gat5"]), W(p["gat6"])
    w7, w8 = W(p["gat7"]), W(p["gat8"])
    wq4, wk4 = W(p["tr4"]["q"]), W(p["tr4"]["k"])

    com = {
        "w1": _bf(W(p["gcn1"])),
        "w2": _bf(W(p["gcn2"])),
        "wq3": _bf(W(p["tr3"]["q"])),
        "wk3": _bf(W(p["tr3"]["k"])),
        "wv3": _bf(W(p["tr3"]["v"])),
        "ws3": _bf(W(p["tr3"]["skip"])),
        "blin4": _bf(wk4 @ wq4.T),
        "wv4": _bf(W(p["tr4"]["v"])),
        "ws4": _bf(W(p["tr4"]["skip"])),
        "w5": _bf(w5),
        "w6": _bf(w6),
        "w7": _bf(w7),
        "w8": _bf(w8),
        "a5s": _bf(_f32(p["gat5"]["a_src"]).reshape(D2, 1)),
        "a5d": _bf(_f32(p["gat5"]["a_dst"]).reshape(D2, 1)),
        "a7s": _bf(_f32(p["gat7"]["a_src"]).reshape(D2, 1)),
        "a7d": _bf(_f32(p["gat7"]["a_dst"]).reshape(D2, 1)),
        "a6s": _f32(w6 @ _f32(p["gat6"]["a_src"])).reshape(1, D2),
        "a6d": _f32(w6 @ _f32(p["gat6"]["a_dst"])).reshape(1, D2),
        "a8s": _f32(w8 @ _f32(p["gat8"]["a_src"])).reshape(1, D2),
        "a8d": _f32(w8 @ _f32(p["gat8"]["a_dst"])).reshape(1, D2),
    }

    in_maps = []
    for c in range(NCORES):
        r0, r1 = c * NL, (c + 1) * NL
        m = dict(com)
        m["xT"] = _bf(x[r0:r1].T)
        m["agcnT"] = _bf(A[r0:r1, :].T)
        m["mslT"] = _bf(Msl[r0:r1, :].T)
        m["mnoslT"] = _bf(Mnosl[r0:r1, :].T)
        in_maps.append(m)
    return in_maps


# ----------------------------------------------------------------------------
# Device program
# ----------------------------------------------------------------------------


def _build(nlayers=8, debug=False):
    nc = bacc.Bacc("TRN2", target_bir_lowering=False, debug=False, num_devices=NCORES)

    def din(name, shape, dt=BF):
        return nc.dram_tensor(name, shape, dt, kind="ExternalInput")

    xT_d = din("xT", [D1, NL])
    agcnT_d = din("agcnT", [N, NL])
    mslT_d = din("mslT", [N, NL])
    mnoslT_d = din("mnoslT", [N, NL])
    w1_d = din("w1", [D1, D2])
    w2_d = din("w2", [D2, D1])
    wq3_d = din("wq3", [D1, D2])
    wk3_d = din("wk3", [D1, D2])
    wv3_d = din("wv3", [D1, D2])
    ws3_d = din("ws3", [D1, D2])
    blin4_d = din("blin4", [D2, D2])
    wv4_d = din("wv4", [D2, D1])
    ws4_d = din("ws4", [D2, D1])
    w5_d = din("w5", [D1, D2])
    w6_d = din("w6", [D2, D1])
    w7_d = din("w7", [D1, D2])
    w8_d = din("w8", [D2, D1])
    a5s_d = din("a5s", [D2, 1])
    a5d_d = din("a5d", [D2, 1])
    a7s_d = din("a7s", [D2, 1])
    a7d_d = din("a7d", [D2, 1])
    a6s_d = din("a6s", [1, D2], FP32)
    a6d_d = din("a6d", [1, D2], FP32)
    a8s_d = din("a8s", [1, D2], FP32)
    a8d_d = din("a8d", [1, D2], FP32)

    out_d = {}
    for k in ("out2", "out4", "out6", "out8"):
        out_d[k] = nc.dram_tensor(k, [D1, NL], FP32, kind="ExternalOutput")
    taps = {}
    if debug:
        for tname in ("tap_h1", "tap_h3", "tap_h5", "tap_h7"):
            taps[tname] = nc.dram_tensor(tname, [NL, C], BF, kind="ExternalOutput")

    def _body(tc, ctx):
        hfull = ctx.enter_context(tc.tile_pool(name="hfull", bufs=2))
        mstream = ctx.enter_context(tc.tile_pool(name="mstream", bufs=6))
        arst = ctx.enter_context(tc.tile_pool(name="arst", bufs=4))
        wpool = ctx.enter_context(tc.tile_pool(name="wpool", bufs=2))
        actp = ctx.enter_context(tc.tile_pool(name="actp", bufs=2))
        sp = ctx.enter_context(tc.tile_pool(name="sp", bufs=2))
        colp = ctx.enter_context(tc.tile_pool(name="colp", bufs=4))
        rowp = ctx.enter_context(tc.tile_pool(name="rowp", bufs=2))
        constp = ctx.enter_context(tc.tile_pool(name="constp", bufs=1))
        dramp = ctx.enter_context(tc.tile_pool(name="dramp", bufs=1, space="DRAM"))
        pacc = ctx.enter_context(tc.tile_pool(name="pacc", bufs=4, space="PSUM"))
        pwork = ctx.enter_context(tc.tile_pool(name="pwork", bufs=2, space="PSUM"))
        psrow = ctx.enter_context(tc.tile_pool(name="psrow", bufs=2, space="PSUM"))

        ident_bf = constp.tile([128, 128], BF, name="ident_bf", tag="idbf")
        cmasks.make_identity(nc, ident_bf[:])
        ident_f = constp.tile([128, 128], FP32, name="ident_f", tag="idf")
        cmasks.make_identity(nc, ident_f[:])
        ones_col_bf = constp.tile([128, 1], BF, name="ones_col_bf", tag="ocb")
        nc.vector.memset(ones_col_bf[:], 1.0)
        ones_row_f = constp.tile([1, 128], FP32, name="ones_row_f", tag="orf")
        nc.vector.memset(ones_row_f[:], 1.0)

        def dma_in_tiled(sb, dram_h, R, cols):
            ap = dram_h.ap() if hasattr(dram_h, "ap") else dram_h
            for t in range(R // 128):
                nc.sync.dma_start(
                    sb[:, t * cols : (t + 1) * cols], ap[t * 128 : (t + 1) * 128, :]
                )

        def load_w(name, dram_h, K, M):
            t = wpool.tile([128, (K // 128) * M], BF, name=name, tag="w", bufs=3)
            dma_in_tiled(t, dram_h, K, M)
            return t

        def MM(ps, lhsT, rhs, first, last):
            nc.tensor.matmul(ps, lhsT, rhs, start=first, stop=last)

        def wmm(lhsT_sb, M, Kt, rhs_sb, evict, rC=C):
            """psum[mt] = sum_kt lhsT[kt, mt-cols]^T @ rhs[kt]; evict(mt, ps)."""
            for mt in range(M // 128):
                ps = pwork.tile([128, rC], FP32, name="mmps", tag="wk", bufs=2)
                for kt in range(Kt):
                    MM(
                        ps[:],
                        lhsT_sb[:, kt * M + mt * 128 : kt * M + (mt + 1) * 128],
                        rhs_sb[:, kt * rC : (kt + 1) * rC],
                        kt == 0,
                        kt == Kt - 1,
                    )
                evict(mt, ps)

        def transpose_blocks(dst_sb, dst_C, src_sb, src_C, Ti, Tj, dt):
            ident = ident_bf if dt == BF else ident_f
            for i in range(Ti):
                for j in range(Tj):
                    pt = pwork.tile([128, 128], dt, name="trps", tag="wk", bufs=2)
                    nc.tensor.transpose(
                        pt[:], src_sb[:, i * src_C + j * 128 : i * src_C + (j + 1) * 128], ident[:]
                    )
                    nc.vector.tensor_copy(
                        dst_sb[:, j * dst_C + i * 128 : j * dst_C + (i + 1) * 128], pt[:]
                    )

        def allgather(src_sb, R, cols, dt, name):
            bi = dramp.tile([R, cols], dt, name=f"agi_{name}", tag=f"agi_{name}")
            bo = dramp.tile([NCORES * R, cols], dt, name=f"ago_{name}", tag=f"ago_{name}")
            for t in range(R // 128):
                nc.sync.dma_start(
                    bi[t * 128 : (t + 1) * 128, :], src_sb[:, t * cols : (t + 1) * cols]
                )
            nc.gpsimd.collective_compute(
                "AllGather", OP.bypass, replica_groups=RG, ins=[bi.opt()], outs=[bo.opt()]
            )
            return bo

        def load_full(bo, name):
            t = hfull.tile([128, TN * C], BF, name=name, tag="hfull", bufs=2)
            for tt in range(TN):
                nc.sync.dma_start(
                    t[:, tt * C : (tt + 1) * C], bo[tt * 128 : (tt + 1) * 128, :]
                )
            return t

        def out_evict(dram_h, mt, ps, bf_dst=None):
            stage = actp.tile([128, NL], FP32, name="outstg", tag="outstg", bufs=2)
            nc.vector.tensor_copy(stage[:], ps[:])
            if bf_dst is not None:
                nc.scalar.activation(bf_dst[:, mt * NL : (mt + 1) * NL], ps[:], AF.Copy)
            nc.sync.dma_start(dram_h.ap()[mt * 128 : (mt + 1) * 128, :], stage[:])

        def elu_from(dst_bf, src):
            m0 = sp.tile([128, C], FP32, name="elu_m", tag="elu_m", bufs=2)
            nc.vector.tensor_scalar_min(m0[:], src, 0.0)
            e0 = sp.tile([128, C], FP32, name="elu_e", tag="elu_e", bufs=2)
            nc.scalar.activation(e0[:], m0[:], AF.Exp)
            nc.vector.scalar_tensor_tensor(dst_bf, e0[:], 1.0, src, OP.subtract, OP.max)

        # ---------------- kt-outer streamed aggregation core ----------------

        def agg_run(ar_producer, hf, node_major, want_s):
            """accs[m] [128,C] psum accumulated over 32 streamed src tiles."""
            accs = [
                pacc.tile([128, C], FP32, name=f"acc{m}", tag="acc", bufs=4)
                for m in range(TL)
            ]
            srow = None
            if want_s:
                srow = psrow.tile([1, C], FP32, name="srow", tag="srow", bufs=2)
            for kt in range(TN):
                ar = ar_producer(kt)
                for m in range(TL):
                    if node_major:
                        MM(
                            accs[m][:],
                            ar[:, m * 128 : (m + 1) * 128],
                            hf[:, kt * C : (kt + 1) * C],
                            kt == 0,
                            kt == TN - 1,
                        )
                    else:
                        MM(
                            accs[m][:],
                            hf[:, kt * C + m * 128 : kt * C + (m + 1) * 128],
                            ar[:],
                            kt == 0,
                            kt == TN - 1,
                        )
                if want_s:
                    MM(srow[:], ones_col_bf[:], ar[:], kt == 0, kt == TN - 1)
            return accs, srow

        def stream_producer(dram_h):
            """GCN: the adjacency tile itself is the attention tile."""

            def prod(kt):
                mt = mstream.tile([128, C], BF, name="mst", tag="mst", bufs=6)
                nc.sync.dma_start(mt[:], dram_h.ap()[kt * 128 : (kt + 1) * 128, :])
                return mt

            return prod

        def gat_producer(mask_d, adstb, asf):
            def prod(kt):
                mt = mstream.tile([128, C], BF, name="mst", tag="mst", bufs=6)
                nc.sync.dma_start(mt[:], mask_d.ap()[kt * 128 : (kt + 1) * 128, :])
                t = sp.tile([128, C], FP32, name="e_t", tag="e_t", bufs=3)
                nc.vector.tensor_scalar_add(t[:], adstb[:], asf[:, kt : kt + 1])
                e1 = sp.tile([128, C], FP32, name="e1", tag="e1", bufs=3)
                nc.vector.scalar_tensor_tensor(e1[:], t[:], 0.2, t[:], OP.mult, OP.max)
                e2 = sp.tile([128, C], BF, name="e2", tag="e2", bufs=3)
                nc.scalar.activation(e2[:], e1[:], AF.Exp)
                ar = arst.tile([128, C], BF, name="ar", tag="ar", bufs=4)
                nc.vector.tensor_mul(ar[:], e2[:], mt[:])
                return ar

            return prod

        def tr_producer(mask_d, lhs_full, rhs_T, scale):
            """logits tile via PE: [128 src, C dst] = sum_kf lhsf^T @ rhs_T."""

            def prod(kt):
                mt = mstream.tile([128, C], BF, name="mst", tag="mst", bufs=6)
                nc.sync.dma_start(mt[:], mask_d.ap()[kt * 128 : (kt + 1) * 128, :])
                r, jo = kt // TL, (kt % TL) * 128
                ps = pwork.tile([128, C], FP32, name="lgps", tag="wk", bufs=2)
                for kf in range(TL):
                    MM(
                        ps[:],
                        lhs_full[:, (r * TL + kf) * C + jo : (r * TL + kf) * C + jo + 128],
                        rhs_T[:, kf * NL : (kf + 1) * NL],
                        kf == 0,
                        kf == TL - 1,
                    )
                e2 = sp.tile([128, C], BF, name="e2", tag="e2", bufs=3)
                nc.scalar.activation(e2[:], ps[:], AF.Exp, scale=scale)
                ar = arst.tile([128, C], BF, name="ar", tag="ar", bufs=4)
                nc.vector.tensor_mul(ar[:], e2[:], mt[:])
                return ar

            return prod

        # ------------- softmax-denominator epilogue helpers -------------

        def srow_to_rb(srow):
            s0 = rowp.tile([1, C], FP32, name="srow_sb", tag="srow_sb", bufs=1)
            nc.vector.tensor_scalar_add(s0[:], srow[:], 1e-16)
            r0 = rowp.tile([1, C], FP32, name="rrow_sb", tag="rrow_sb", bufs=1)
            nc.vector.reciprocal(r0[:], s0[:])
            pb = pwork.tile([128, C], FP32, name="bcps", tag="wk", bufs=2)
            MM(pb[:], ones_row_f[:], r0[:], True, True)
            rb = sp.tile([128, C], FP32, name="rb", tag="rb", bufs=1)
            nc.vector.tensor_copy(rb[:], pb[:])
            return rb

        def srow_to_cols(srow):
            s0 = rowp.tile([1, C], FP32, name="srow_sb", tag="srow_sb", bufs=1)
            nc.vector.tensor_scalar_add(s0[:], srow[:], 1e-16)
            r0 = rowp.tile([1, C], FP32, name="rrow_sb", tag="rrow_sb", bufs=1)
            nc.vector.reciprocal(r0[:], s0[:])
            cols = colp.tile([128, TL], FP32, name="rcols", tag="rcols", bufs=2)
            for j in range(TL):
                # [1,128] row -> [128,1] col reshape via small SBUF->SBUF DMA
                nc.sync.dma_start(
                    cols[:, j : j + 1], r0[0:1, j * 128 : (j + 1) * 128]
                )
            return cols

        def attn_cols_from_hT(hT_sb, avec_sb, Kt):
            cols = colp.tile([128, TL], FP32, name="acols", tag="acols", bufs=4)
            for mn in range(TL):
                psc = pwork.tile([128, 1], FP32, name="acps", tag="wk", bufs=2)
                for kf in range(Kt):
                    MM(
                        psc[:],
                        hT_sb[:, kf * NL + mn * 128 : kf * NL + (mn + 1) * 128],
                        avec_sb[:, kf : kf + 1],
                        kf == 0,
                        kf == Kt - 1,
                    )
                nc.vector.tensor_copy(cols[:, mn : mn + 1], psc[:])
            return cols

        def attn_cols_dve(h_node_sb, ab_sb):
            cols = colp.tile([128, TL], FP32, name="dcols", tag="acols", bufs=4)
            for mn in range(TL):
                junk = sp.tile([128, C], FP32, name="ttr_junk", tag="junk", bufs=2)
                nc.vector.tensor_mul(junk[:], h_node_sb[:, mn * C : (mn + 1) * C], ab_sb[:])
                nc.vector.reduce_sum(cols[:, mn : mn + 1], junk[:], axis=mybir.AxisListType.X)
            return cols

        def cols_to_bcast(cols):
            row = rowp.tile([1, C], FP32, name="adrow", tag="adrow", bufs=1)
            for j in range(TL):
                pt = pwork.tile([1, 128], FP32, name="trrow", tag="wk", bufs=2)
                nc.tensor.transpose(pt[:], cols[:, j : j + 1], ident_f[:])
                nc.vector.tensor_copy(row[0:1, j * 128 : (j + 1) * 128], pt[:])
            pb = pwork.tile([128, C], FP32, name="adbc", tag="wk", bufs=2)
            MM(pb[:], ones_row_f[:], row[:], True, True)
            out = sp.tile([128, C], FP32, name="adstb", tag="adstb", bufs=1)
            nc.vector.tensor_copy(out[:], pb[:])
            return out

        def row_to_bcast_bf(row_sb):
            pb = pwork.tile([128, C], FP32, name="abps", tag="wk", bufs=2)
            MM(pb[:], ones_row_f[:], row_sb[:], True, True)
            out = sp.tile([128, C], BF, name="ab", tag="ab", bufs=2)
            nc.vector.tensor_copy(out[:], pb[:])
            return out

        def asrc_allgather(cols, name):
            bi = dramp.tile([NL, 1], FP32, name=f"agi_{name}", tag=f"agi_{name}")
            bo = dramp.tile([N, 1], FP32, name=f"ago_{name}", tag=f"ago_{name}")
            for t in range(TL):
                nc.sync.dma_start(bi[t * 128 : (t + 1) * 128, :], cols[:, t : t + 1])
            nc.gpsimd.collective_compute(
                "AllGather", OP.bypass, replica_groups=RG, ins=[bi.opt()], outs=[bo.opt()]
            )
            asf = colp.tile([128, TN], FP32, name=f"asf_{name}", tag="asf", bufs=2)
            nc.sync.dma_start(
                asf[:].rearrange("p (t c) -> p t c", c=1),
                bo.rearrange("(t p) c -> p t c", p=128),
            )
            return asf

        def dma_tap(name, sb):
            if debug and name in taps:
                for t in range(TL):
                    nc.sync.dma_start(
                        taps[name].ap()[t * 128 : (t + 1) * 128, :], sb[:, t * C : (t + 1) * C]
                    )

        # ==================== L1: GCN1 (1024 -> 512), ELU ====================
        with nc.named_scope("L1_gcn1"):
            xT_sb = actp.tile([128, (D1 // 128) * NL], BF, name="xT_sb", tag="hT_in", bufs=2)
            dma_in_tiled(xT_sb, xT_d, D1, NL)
            w1 = load_w("w1_sb", w1_d, D1, D2)
            xw1T = actp.tile([128, TL * C], BF, name="xw1T", tag="pT", bufs=1)
            wmm(
                w1,
                D2,
                D1 // 128,
                xT_sb,
                lambda mt, ps: nc.vector.tensor_copy(xw1T[:, mt * NL : (mt + 1) * NL], ps[:]),
                rC=NL,
            )
            xw1 = actp.tile([128, TL * C], BF, name="xw1", tag="hN", bufs=2)
            transpose_blocks(xw1, C, xw1T, NL, TL, TL, BF)
            bo1 = allgather(xw1, NL, C, BF, "xw1")
            xw1f = load_full(bo1, "xw1full")

            h1 = actp.tile([128, TL * C], BF, name="h1", tag="hN", bufs=2)
            accs, _ = agg_run(stream_producer(agcnT_d), xw1f, True, False)
            for mn in range(TL):
                elu_from(h1[:, mn * C : (mn + 1) * C], accs[mn][:])
            dma_tap("tap_h1", h1)

        if nlayers < 2:
            return

        # ==================== L2: GCN2 (512 -> 1024) ====================
        with nc.named_scope("L2_gcn2"):
            bo2 = allgather(h1, NL, C, BF, "h1")
            h1f = load_full(bo2, "h1full")
            agT = actp.tile([128, TL * C], BF, name="agT", tag="pT", bufs=1)
            accs, _ = agg_run(stream_producer(agcnT_d), h1f, False, False)
            for mf in range(TL):
                nc.vector.tensor_copy(agT[:, mf * C : (mf + 1) * C], accs[mf][:])
            w2 = load_w("w2_sb", w2_d, D2, D1)
            h2T_b = actp.tile([128, (D1 // 128) * NL], BF, name="h2T_b", tag="hT_in", bufs=2)

            def ev2(mt, ps):
                out_evict(out_d["out2"], mt, ps, bf_dst=h2T_b)

            wmm(w2, D1, D2 // 128, agT, ev2, rC=NL)

        if nlayers < 3:
            return

        # ==================== L3: TransformerConv (1024 -> 512), ELU ====================
        with nc.named_scope("L3_tr3"):
            wq3 = load_w("wq3_sb", wq3_d, D1, D2)
            wk3 = load_w("wk3_sb", wk3_d, D1, D2)
            wv3 = load_w("wv3_sb", wv3_d, D1, D2)
            ws3 = load_w("ws3_sb", ws3_d, D1, D2)

            qT = actp.tile([128, TL * C], BF, name="qT", tag="qT", bufs=1)
            wmm(wq3, D2, D1 // 128, h2T_b, lambda mt, ps: nc.vector.tensor_copy(qT[:, mt * NL : (mt + 1) * NL], ps[:]), rC=NL)
            kT = actp.tile([128, TL * C], BF, name="kT", tag="kT", bufs=1)
            wmm(wk3, D2, D1 // 128, h2T_b, lambda mt, ps: nc.vector.tensor_copy(kT[:, mt * NL : (mt + 1) * NL], ps[:]), rC=NL)
            vT = actp.tile([128, TL * C], BF, name="vT", tag="pT", bufs=1)
            wmm(wv3, D2, D1 // 128, h2T_b, lambda mt, ps: nc.vector.tensor_copy(vT[:, mt * NL : (mt + 1) * NL], ps[:]), rC=NL)
            skT = actp.tile([128, TL * C], FP32, name="skT", tag="skT", bufs=1)
            wmm(ws3, D2, D1 // 128, h2T_b, lambda mt, ps: nc.vector.tensor_copy(skT[:, mt * NL : (mt + 1) * NL], ps[:]), rC=NL)

            bo_k = allgather(kT, D2, NL, BF, "kT3")
            kTf = load_full(bo_k, "kTfull3")
            v3 = actp.tile([128, TL * C], BF, name="v3", tag="hN", bufs=2)
            transpose_blocks(v3, C, vT, NL, TL, TL, BF)
            bo_v = allgather(v3, NL, C, BF, "v3")
            vf = load_full(bo_v, "vfull3")

            accs, srow = agg_run(
                tr_producer(mnoslT_d, kTf, qT, float(1.0 / np.sqrt(D2))), vf, False, True
            )
            rb = srow_to_rb(srow)
            h3T = actp.tile([128, TL * C], BF, name="h3T", tag="hT_in2", bufs=1)
            for mf in range(TL):
                t0 = sp.tile([128, C], FP32, name="t0_3", tag="t0", bufs=2)
                nc.vector.tensor_mul(t0[:], accs[mf][:], rb[:])
                nc.vector.tensor_add(t0[:], t0[:], skT[:, mf * NL : (mf + 1) * NL])
                elu_from(h3T[:, mf * NL : (mf + 1) * NL], t0[:])
            h3 = actp.tile([128, TL * C], BF, name="h3", tag="hN", bufs=2)
            transpose_blocks(h3, C, h3T, NL, TL, TL, BF)
            dma_tap("tap_h3", h3)

        if nlayers < 4:
            return

        # ==================== L4: TransformerConv (512 -> 1024) ====================
        with nc.named_scope("L4_tr4"):
            bo_h3 = allgather(h3, NL, C, BF, "h3")
            blin4 = load_w("blin4_sb", blin4_d, D2, D2)
            GT = actp.tile([128, TL * C], BF, name="GT", tag="pT", bufs=1)
            wmm(blin4, D2, D2 // 128, h3T, lambda mt, ps: nc.vector.tensor_copy(GT[:, mt * NL : (mt + 1) * NL], ps[:]), rC=NL)
            bo_g = allgather(GT, D2, NL, BF, "GT4")
            GTf = load_full(bo_g, "GTfull4")
            h3f = load_full(bo_h3, "h3full")

            accs, srow = agg_run(
                tr_producer(mnoslT_d, GTf, h3T, float(1.0 / np.sqrt(D1))), h3f, False, True
            )
            rb = srow_to_rb(srow)
            agTn = actp.tile([128, TL * C], BF, name="agTn", tag="agT", bufs=1)
            for mf in range(TL):
                nc.vector.tensor_mul(agTn[:, mf * C : (mf + 1) * C], accs[mf][:], rb[:])

            wv4 = load_w("wv4_sb", wv4_d, D2, D1)
            ws4 = load_w("ws4_sb", ws4_d, D2, D1)
            h4T_b = actp.tile([128, (D1 // 128) * NL], BF, name="h4T_b", tag="hT_in", bufs=2)
            for mt in range(D1 // 128):
                ps = pwork.tile([128, NL], FP32, name="mm4", tag="wk", bufs=2)
                for kt in range(D2 // 128):
                    MM(
                        ps[:],
                        wv4[:, kt * D1 + mt * 128 : kt * D1 + (mt + 1) * 128],
                        agTn[:, kt * C : (kt + 1) * C],
                        kt == 0,
                        False,
                    )
                for kt in range(D2 // 128):
                    MM(
                        ps[:],
                        ws4[:, kt * D1 + mt * 128 : kt * D1 + (mt + 1) * 128],
                        h3T[:, kt * NL : (kt + 1) * NL],
                        False,
                        kt == D2 // 128 - 1,
                    )
                out_evict(out_d["out4"], mt, ps, bf_dst=h4T_b)

        if nlayers < 5:
            return

        # ==================== GAT layers ====================
        def gat_project_first(lname, hT_in, w_d_, as_d, ad_d, mask_d, tapname):
            w_sb = load_w(f"w_{lname}", w_d_, D1, D2)
            hpT = actp.tile([128, TL * C], BF, name=f"hpT_{lname}", tag="pT", bufs=1)
            wmm(
                w_sb,
                D2,
                D1 // 128,
                hT_in,
                lambda mt, ps: nc.vector.tensor_copy(hpT[:, mt * NL : (mt + 1) * NL], ps[:]),
                rC=NL,
            )
            avs = colp.tile([128, TL], BF, name=f"avs_{lname}", tag="av", bufs=4)
            nc.sync.dma_start(
                avs[:].rearrange("p (k c) -> p k c", c=1),
                as_d.ap().rearrange("(k p) c -> p k c", p=128),
            )
            avd = colp.tile([128, TL], BF, name=f"avd_{lname}", tag="av", bufs=4)
            nc.sync.dma_start(
                avd[:].rearrange("p (k c) -> p k c", c=1),
                ad_d.ap().rearrange("(k p) c -> p k c", p=128),
            )
            asrc_cols = attn_cols_from_hT(hpT, avs, TL)
            adst_cols = attn_cols_from_hT(hpT, avd, TL)
            asf = asrc_allgather(asrc_cols, f"as_{lname}")
            adstb = cols_to_bcast(adst_cols)

            hp = actp.tile([128, TL * C], BF, name=f"hp_{lname}", tag="hN", bufs=2)
            transpose_blocks(hp, C, hpT, NL, TL, TL, BF)
            bo = allgather(hp, NL, C, BF, f"hp_{lname}")
            hpf = load_full(bo, f"hpf_{lname}")

            accs, srow = agg_run(gat_producer(mask_d, adstb, asf), hpf, True, True)
            rcols = srow_to_cols(srow)
            h_out = actp.tile([128, TL * C], BF, name=f"hout_{lname}", tag="hN", bufs=2)
            for mn in range(TL):
                t0 = sp.tile([128, C], FP32, name=f"t0_{lname}", tag="t0", bufs=2)
                nc.vector.tensor_scalar_mul(t0[:], accs[mn][:], rcols[:, mn : mn + 1])
                elu_from(h_out[:, mn * C : (mn + 1) * C], t0[:])
            dma_tap(tapname, h_out)
            return h_out

        def gat_agg_first(lname, h_node, as_row_d, ad_row_d, w_d_, mask_d, out_name, next_bf):
            r_s = rowp.tile([1, C], FP32, name=f"ars_{lname}", tag="ar_row", bufs=2)
            nc.sync.dma_start(r_s[:], as_row_d.ap())
            r_d = rowp.tile([1, C], FP32, name=f"ard_{lname}", tag="ar_row", bufs=2)
            nc.sync.dma_start(r_d[:], ad_row_d.ap())
            ab_s = row_to_bcast_bf(r_s)
            ab_d = row_to_bcast_bf(r_d)

            asrc_cols = attn_cols_dve(h_node, ab_s)
            adst_cols = attn_cols_dve(h_node, ab_d)
            asf = asrc_allgather(asrc_cols, f"as_{lname}")
            adstb = cols_to_bcast(adst_cols)

            bo = allgather(h_node, NL, C, BF, f"h_{lname}")
            hf = load_full(bo, f"hf_{lname}")

            accs, srow = agg_run(gat_producer(mask_d, adstb, asf), hf, False, True)
            rb = srow_to_rb(srow)
            agTn_ = actp.tile([128, TL * C], BF, name=f"agTn_{lname}", tag="agT", bufs=1)
            for mf in range(TL):
                nc.vector.tensor_mul(agTn_[:, mf * C : (mf + 1) * C], accs[mf][:], rb[:])

            w_sb = load_w(f"w_{lname}", w_d_, D2, D1)
            hT_b = None
            if next_bf:
                hT_b = actp.tile([128, (D1 // 128) * NL], BF, name=f"hTb_{lname}", tag="hT_in", bufs=2)

            def evw(mt, ps):
                out_evict(out_d[out_name], mt, ps, bf_dst=hT_b)

            wmm(w_sb, D1, D2 // 128, agTn_, evw, rC=NL)
            return hT_b

        with nc.named_scope("L5_gat5"):
            h5 = gat_project_first("g5", h4T_b, w5_d, a5s_d, a5d_d, mslT_d, "tap_h5")
        if nlayers < 6:
            return
        with nc.named_scope("L6_gat6"):
            h6T_b = gat_agg_first("g6", h5, a6s_d, a6d_d, w6_d, mslT_d, "out6", next_bf=True)
        if nlayers < 7:
            return
        with nc.named_scope("L7_gat7"):
            h7 = gat_project_first("g7", h6T_b, w7_d, a7s_d, a7d_d, mslT_d, "tap_h7")
        if nlayers < 8:
            return
        with nc.named_scope("L8_gat8"):
            gat_agg_first("g8", h7, a8s_d, a8d_d, w8_d, mslT_d, "out8", next_bf=False)

    with tile.TileContext(nc) as tc, ExitStack() as ctx:
        _body(tc, ctx)

    nc.compile()
    nc.m = get_hw_module(nc.m)
    return nc


# ----------------------------------------------------------------------------
# Runner
# ----------------------------------------------------------------------------


def _install_profile_shim():
    import types
    from trn_agent_boot.trn_boot import _ntff_profile_via_ctypes

    if "antenv.axon_hooks" in sys.modules:
        return
    try:
        hook = _ntff_profile_via_ctypes("/opt/axon/libaxon_pjrt.so")
    except OSError:
        hook = None
    mod = types.ModuleType("antenv.axon_hooks")
    mod.get_axon_ntff_profile_hook = lambda: hook
    sys.modules["antenv.axon_hooks"] = mod
    bass_utils.upload_artifacts = lambda tmpdir: "local://" + tmpdir


def _get_program(nlayers=8, debug=False):
    key = (nlayers, debug)
    if key not in _CACHE:
        _CACHE[key] = _build(nlayers=nlayers, debug=debug)
    return _CACHE[key]


def _run(inputs, trace=False, nlayers=8, debug=False):
    nc = _get_program(nlayers=nlayers, debug=debug)
    in_maps = _host_prep(inputs["x"], inputs["edge_index"], inputs["params"])
    if trace:
        _install_profile_shim()
    res = bass_utils.run_bass_kernel_spmd(
        nc, in_maps, core_ids=list(range(NCORES)), trace=trace
    )
    return res


def kernel(x, edge_index, params):
    res = _run({"x": x, "edge_index": edge_index, "params": params}, trace=False)
    outs = []
    for name in ("out2", "out4", "out6", "out8"):
        full = np.concatenate(
            [np.asarray(res.results[c][name]).T for c in range(NCORES)], axis=0
        )
        outs.append(full[None].astype(np.float32))
    return tuple(outs)


# revision 18
# speedup vs baseline: 1.0658x; 1.0658x over previous
"""Trainium2 Bass kernel for nn_GAT_Mixer (8-layer GNN: GCN x2, TransformerConv x2, GAT x4).

Strategy (8 NeuronCores, node-sharded, dense-adjacency formulation):
  - Nodes row-sharded: core c owns rows [512c, 512(c+1)).
  - All segment ops become dense matmuls against a [4096 src, 512 dst_local]
    column slice of the (transposed) adjacency/attention matrix per core.
  - GCN: sym-normalized adjacency precomputed dense on host (bf16), streamed.
  - GAT/Transformer: per-edge softmax == masked dense softmax; mask = edge
    multiplicity matrix (host, bf16, streamed per 128-src-row tile).
    araw[s,d] = mult[s,d]*exp(logit[s,d]); out = (araw^T @ h)/rowsum.
  - Aggregation is kt-outer streamed: for each 128-src tile, build the araw
    tile (DVE/ACT) and immediately accumulate into 4 persistent PSUM banks
    (+1 rowsum rider bank) so TensorE stays busy and HAM-warm.
  - All aggregations contract at feature dim 512 via A@(xW) = (A@x)W
    reordering; GAT attention vectors folded a' = W@a on host; tr4 logits use
    the bilinear trick q.k = x (Wk Wq^T) x^T at dim 512.
  - Cross-core: AllGather of 512-row local activations (bf16) per layer.

kernel(**inputs) takes FULL inputs, returns (h2, h4, h6, h8) each
[1, 4096, 1024] float32, matching reference().
"""

import os
import sys

sys.path.insert(0, "/opt/trn_rl_repo")
if "/root/.axon_site" not in sys.path:
    sys.path.insert(0, "/root/.axon_site")

import numpy as np
import ml_dtypes
from contextlib import ExitStack

import concourse.bass as bass
import concourse.mybir as mybir
import concourse.tile as tile
from concourse import bacc
from concourse import bass_utils
from concourse import masks as cmasks
from concourse.bass_interp import get_hw_module

BF16 = ml_dtypes.bfloat16
FP32 = mybir.dt.float32
BF = mybir.dt.bfloat16
AF = mybir.ActivationFunctionType
OP = mybir.AluOpType

NCORES = 8
N = 4096
NL = N // NCORES  # 512 local nodes per core
D1 = 1024
D2 = 512
TN = N // 128  # 32 global node tiles
TL = NL // 128  # 4 local node tiles
C = 512  # canonical free-dim width
RG = [list(range(NCORES))]

_CACHE = {}


def _bf(a):
    return np.ascontiguousarray(np.asarray(a, dtype=np.float32).astype(BF16))


def _f32(a):
    return np.ascontiguousarray(np.asarray(a), dtype=np.float32)


# ----------------------------------------------------------------------------
# Host-side preprocessing
# ----------------------------------------------------------------------------


def _host_prep(x, edge_index, params):
    x = _f32(x)
    ei = np.asarray(edge_index)
    src = ei[0].astype(np.int64)
    dst = ei[1].astype(np.int64)
    loop = np.arange(N, dtype=np.int64)
    src_sl = np.concatenate([src, loop])
    dst_sl = np.concatenate([dst, loop])

    deg = np.zeros(N, dtype=np.float32)
    np.add.at(deg, dst_sl, 1.0)
    dinv = np.where(deg > 0, np.maximum(deg, 1.0) ** -0.5, 0.0).astype(np.float32)
    A = np.zeros((N, N), dtype=np.float32)
    np.add.at(A, (dst_sl, src_sl), dinv[src_sl] * dinv[dst_sl])

    Msl = np.zeros((N, N), dtype=np.float32)
    np.add.at(Msl, (dst_sl, src_sl), 1.0)
    Mnosl = np.zeros((N, N), dtype=np.float32)
    np.add.at(Mnosl, (dst, src), 1.0)

    p = params

    def W(q):
        return _f32(q["W"])

    def b(q):
        return _f32(q["b"])

    for name in ("gcn1", "gcn2", "gat5", "gat6", "gat7", "gat8"):
        assert not np.any(b(p[name])), f"nonzero bias {name} unsupported"
    for name in ("tr3", "tr4"):
        for sub in ("q", "k", "v", "skip"):
            assert not np.any(b(p[name][sub])), f"nonzero bias {name}.{sub}"

    w5, w6 = W(p["gat5"]), W(p["gat6"])
    w7, w8 = W(p["gat7"]), W(p["gat8"])
    wq4, wk4 = W(p["tr4"]["q"]), W(p["tr4"]["k"])

    com = {
        "w1": _bf(W(p["gcn1"])),
        "w2": _bf(W(p["gcn2"])),
        "wq3": _bf(W(p["tr3"]["q"])),
        "wk3": _bf(W(p["tr3"]["k"])),
        "wv3": _bf(W(p["tr3"]["v"])),
        "ws3": _bf(W(p["tr3"]["skip"])),
        "blin4": _bf(wk4 @ wq4.T),
        "wv4": _bf(W(p["tr4"]["v"])),
        "ws4": _bf(W(p["tr4"]["skip"])),
        "w5": _bf(w5),
        "w6": _bf(w6),
        "w7": _bf(w7),
        "w8": _bf(w8),
        "a5s": _bf(_f32(p["gat5"]["a_src"]).reshape(D2, 1)),
        "a5d": _bf(_f32(p["gat5"]["a_dst"]).reshape(D2, 1)),
        "a7s": _bf(_f32(p["gat7"]["a_src"]).reshape(D2, 1)),
        "a7d": _bf(_f32(p["gat7"]["a_dst"]).reshape(D2, 1)),
        "a6s": _f32(w6 @ _f32(p["gat6"]["a_src"])).reshape(1, D2),
        "a6d": _f32(w6 @ _f32(p["gat6"]["a_dst"])).reshape(1, D2),
        "a8s": _f32(w8 @ _f32(p["gat8"]["a_src"])).reshape(1, D2),
        "a8d": _f32(w8 @ _f32(p["gat8"]["a_dst"])).reshape(1, D2),
    }

    in_maps = []
    for c in range(NCORES):
        r0, r1 = c * NL, (c + 1) * NL
        m = dict(com)
        m["xT"] = _bf(x[r0:r1].T)
        m["agcnT"] = _bf(A[r0:r1, :].T)
        m["mslT"] = _bf(Msl[r0:r1, :].T)
        m["mnoslT"] = _bf(Mnosl[r0:r1, :].T)
        in_maps.append(m)
    return in_maps


# ----------------------------------------------------------------------------
# Device program
# ----------------------------------------------------------------------------


def _build(nlayers=8, debug=False):
    nc = bacc.Bacc("TRN2", target_bir_lowering=False, debug=False, num_devices=NCORES)

    def din(name, shape, dt=BF):
        return nc.dram_tensor(name, shape, dt, kind="ExternalInput")

    xT_d = din("xT", [D1, NL])
    agcnT_d = din("agcnT", [N, NL])
    mslT_d = din("mslT", [N, NL])
    mnoslT_d = din("mnoslT", [N, NL])
    w1_d = din("w1", [D1, D2])
    w2_d = din("w2", [D2, D1])
    wq3_d = din("wq3", [D1, D2])
    wk3_d = din("wk3", [D1, D2])
    wv3_d = din("wv3", [D1, D2])
    ws3_d = din("ws3", [D1, D2])
    blin4_d = din("blin4", [D2, D2])
    wv4_d = din("wv4", [D2, D1])
    ws4_d = din("ws4", [D2, D1])
    w5_d = din("w5", [D1, D2])
    w6_d = din("w6", [D2, D1])
    w7_d = din("w7", [D1, D2])
    w8_d = din("w8", [D2, D1])
    a5s_d = din("a5s", [D2, 1])
    a5d_d = din("a5d", [D2, 1])
    a7s_d = din("a7s", [D2, 1])
    a7d_d = din("a7d", [D2, 1])
    a6s_d = din("a6s", [1, D2], FP32)
    a6d_d = din("a6d", [1, D2], FP32)
    a8s_d = din("a8s", [1, D2], FP32)
    a8d_d = din("a8d", [1, D2], FP32)

    out_d = {}
    for k in ("out2", "out4", "out6", "out8"):
        out_d[k] = nc.dram_tensor(k, [D1, NL], FP32, kind="ExternalOutput")
    taps = {}
    if debug:
        for tname in ("tap_h1", "tap_h3", "tap_h5", "tap_h7"):
            taps[tname] = nc.dram_tensor(tname, [NL, C], BF, kind="ExternalOutput")

    def _body(tc, ctx):
        hfull = ctx.enter_context(tc.tile_pool(name="hfull", bufs=2))
        mstream = ctx.enter_context(tc.tile_pool(name="mstream", bufs=6))
        arst = ctx.enter_context(tc.tile_pool(name="arst", bufs=4))
        wpool = ctx.enter_context(tc.tile_pool(name="wpool", bufs=2))
        actp = ctx.enter_context(tc.tile_pool(name="actp", bufs=2))
        sp = ctx.enter_context(tc.tile_pool(name="sp", bufs=2))
        colp = ctx.enter_context(tc.tile_pool(name="colp", bufs=4))
        rowp = ctx.enter_context(tc.tile_pool(name="rowp", bufs=2))
        constp = ctx.enter_context(tc.tile_pool(name="constp", bufs=1))
        dramp = ctx.enter_context(tc.tile_pool(name="dramp", bufs=1, space="DRAM"))
        pacc = ctx.enter_context(tc.tile_pool(name="pacc", bufs=4, space="PSUM"))
        pwork = ctx.enter_context(tc.tile_pool(name="pwork", bufs=2, space="PSUM"))
        psrow = ctx.enter_context(tc.tile_pool(name="psrow", bufs=2, space="PSUM"))

        ident_bf = constp.tile([128, 128], BF, name="ident_bf", tag="idbf")
        cmasks.make_identity(nc, ident_bf[:])
        ident_f = constp.tile([128, 128], FP32, name="ident_f", tag="idf")
        cmasks.make_identity(nc, ident_f[:])
        ones_col_bf = constp.tile([128, 1], BF, name="ones_col_bf", tag="ocb")
        nc.vector.memset(ones_col_bf[:], 1.0)
        ones_row_f = constp.tile([1, 128], FP32, name="ones_row_f", tag="orf")
        nc.vector.memset(ones_row_f[:], 1.0)

        def dma_in_tiled(sb, dram_h, R, cols):
            ap = dram_h.ap() if hasattr(dram_h, "ap") else dram_h
            for t in range(R // 128):
                nc.sync.dma_start(
                    sb[:, t * cols : (t + 1) * cols], ap[t * 128 : (t + 1) * 128, :]
                )

        def load_w(name, dram_h, K, M):
            t = wpool.tile([128, (K // 128) * M], BF, name=name, tag="w", bufs=3)
            dma_in_tiled(t, dram_h, K, M)
            return t

        def MM(ps, lhsT, rhs, first, last):
            nc.tensor.matmul(ps, lhsT, rhs, start=first, stop=last)

        def wmm(lhsT_sb, M, Kt, rhs_sb, evict, rC=C):
            """psum[mt] = sum_kt lhsT[kt, mt-cols]^T @ rhs[kt]; evict(mt, ps)."""
            for mt in range(M // 128):
                ps = pwork.tile([128, rC], FP32, name="mmps", tag="wk", bufs=2)
                for kt in range(Kt):
                    MM(
                        ps[:],
                        lhsT_sb[:, kt * M + mt * 128 : kt * M + (mt + 1) * 128],
                        rhs_sb[:, kt * rC : (kt + 1) * rC],
                        kt == 0,
                        kt == Kt - 1,
                    )
                evict(mt, ps)

        def transpose_blocks(dst_sb, dst_C, src_sb, src_C, Ti, Tj, dt):
            ident = ident_bf if dt == BF else ident_f
            for i in range(Ti):
                for j in range(Tj):
                    pt = pwork.tile([128, 128], dt, name="trps", tag="wk", bufs=2)
                    nc.tensor.transpose(
                        pt[:], src_sb[:, i * src_C + j * 128 : i * src_C + (j + 1) * 128], ident[:]
                    )
                    nc.vector.tensor_copy(
                        dst_sb[:, j * dst_C + i * 128 : j * dst_C + (i + 1) * 128], pt[:]
                    )

        def allgather(src_sb, R, cols, dt, name):
            bi = dramp.tile([R, cols], dt, name=f"agi_{name}", tag=f"agi_{name}")
            bo = dramp.tile(
                [NCORES * R, cols], dt, name=f"ago_{name}", tag=f"ago_{name}", addr_space="Shared"
            )
            for t in range(R // 128):
                nc.sync.dma_start(
                    bi[t * 128 : (t + 1) * 128, :], src_sb[:, t * cols : (t + 1) * cols]
                )
            nc.gpsimd.collective_compute(
                "AllGather", OP.bypass, replica_groups=RG, ins=[bi.opt()], outs=[bo.opt()]
            )
            return bo

        def load_full(bo, name):
            t = hfull.tile([128, TN * C], BF, name=name, tag="hfull", bufs=2)
            for tt in range(TN):
                nc.sync.dma_start(
                    t[:, tt * C : (tt + 1) * C], bo[tt * 128 : (tt + 1) * 128, :]
                )
            return t

        def out_evict(dram_h, mt, ps, bf_dst=None):
            stage = actp.tile([128, NL], FP32, name="outstg", tag="outstg", bufs=2)
            nc.vector.tensor_copy(stage[:], ps[:])
            if bf_dst is not None:
                nc.scalar.activation(bf_dst[:, mt * NL : (mt + 1) * NL], ps[:], AF.Copy)
            nc.sync.dma_start(dram_h.ap()[mt * 128 : (mt + 1) * 128, :], stage[:])

        def elu_from(dst_bf, src):
            m0 = sp.tile([128, C], FP32, name="elu_m", tag="elu_m", bufs=2)
            nc.vector.tensor_scalar_min(m0[:], src, 0.0)
            e0 = sp.tile([128, C], FP32, name="elu_e", tag="elu_e", bufs=2)
            nc.scalar.activation(e0[:], m0[:], AF.Exp)
            nc.vector.scalar_tensor_tensor(dst_bf, e0[:], 1.0, src, OP.subtract, OP.max)

        # ---------------- kt-outer streamed aggregation core ----------------

        def agg_run(ar_producer, hf, node_major, want_s):
            """accs[m] [128,C] psum accumulated over 32 streamed src tiles."""
            accs = [
                pacc.tile([128, C], FP32, name=f"acc{m}", tag="acc", bufs=4)
                for m in range(TL)
            ]
            srow = None
            if want_s:
                srow = psrow.tile([1, C], FP32, name="srow", tag="srow", bufs=2)
            for kt in range(TN):
                ar = ar_producer(kt)
                for m in range(TL):
                    if node_major:
                        MM(
                            accs[m][:],
                            ar[:, m * 128 : (m + 1) * 128],
                            hf[:, kt * C : (kt + 1) * C],
                            kt == 0,
                            kt == TN - 1,
                        )
                    else:
                        MM(
                            accs[m][:],
                            hf[:, kt * C + m * 128 : kt * C + (m + 1) * 128],
                            ar[:],
                            kt == 0,
                            kt == TN - 1,
                        )
                if want_s:
                    MM(srow[:], ones_col_bf[:], ar[:], kt == 0, kt == TN - 1)
            return accs, srow

        def stream_producer(dram_h):
            """GCN: the adjacency tile itself is the attention tile."""

            def prod(kt):
                mt = mstream.tile([128, C], BF, name="mst", tag="mst", bufs=6)
                nc.sync.dma_start(mt[:], dram_h.ap()[kt * 128 : (kt + 1) * 128, :])
                return mt

            return prod

        def gat_producer(mask_d, adstb, asf):
            def prod(kt):
                mt = mstream.tile([128, C], BF, name="mst", tag="mst", bufs=6)
                nc.sync.dma_start(mt[:], mask_d.ap()[kt * 128 : (kt + 1) * 128, :])
                t = sp.tile([128, C], FP32, name="e_t", tag="e_t", bufs=3)
                nc.vector.tensor_scalar_add(t[:], adstb[:], asf[:, kt : kt + 1])
                e1 = sp.tile([128, C], FP32, name="e1", tag="e1", bufs=3)
                nc.vector.scalar_tensor_tensor(e1[:], t[:], 0.2, t[:], OP.mult, OP.max)
                e2 = sp.tile([128, C], BF, name="e2", tag="e2", bufs=3)
                nc.scalar.activation(e2[:], e1[:], AF.Exp)
                ar = arst.tile([128, C], BF, name="ar", tag="ar", bufs=4)
                nc.vector.tensor_mul(ar[:], e2[:], mt[:])
                return ar

            return prod

        def tr_producer(mask_d, lhs_full, rhs_T, scale):
            """logits tile via PE: [128 src, C dst] = sum_kf lhsf^T @ rhs_T."""

            def prod(kt):
                mt = mstream.tile([128, C], BF, name="mst", tag="mst", bufs=6)
                nc.sync.dma_start(mt[:], mask_d.ap()[kt * 128 : (kt + 1) * 128, :])
                r, jo = kt // TL, (kt % TL) * 128
                ps = pwork.tile([128, C], FP32, name="lgps", tag="wk", bufs=2)
                for kf in range(TL):
                    MM(
                        ps[:],
                        lhs_full[:, (r * TL + kf) * C + jo : (r * TL + kf) * C + jo + 128],
                        rhs_T[:, kf * NL : (kf + 1) * NL],
                        kf == 0,
                        kf == TL - 1,
                    )
                e2 = sp.tile([128, C], BF, name="e2", tag="e2", bufs=3)
                nc.scalar.activation(e2[:], ps[:], AF.Exp, scale=scale)
                ar = arst.tile([128, C], BF, name="ar", tag="ar", bufs=4)
                nc.vector.tensor_mul(ar[:], e2[:], mt[:])
                return ar

            return prod

        # ------------- softmax-denominator epilogue helpers -------------

        def srow_to_rb(srow):
            s0 = rowp.tile([1, C], FP32, name="srow_sb", tag="srow_sb", bufs=1)
            nc.vector.tensor_scalar_add(s0[:], srow[:], 1e-16)
            r0 = rowp.tile([1, C], FP32, name="rrow_sb", tag="rrow_sb", bufs=1)
            nc.vector.reciprocal(r0[:], s0[:])
            pb = pwork.tile([128, C], FP32, name="bcps", tag="wk", bufs=2)
            MM(pb[:], ones_row_f[:], r0[:], True, True)
            rb = sp.tile([128, C], FP32, name="rb", tag="rb", bufs=1)
            nc.vector.tensor_copy(rb[:], pb[:])
            return rb

        def srow_to_cols(srow):
            s0 = rowp.tile([1, C], FP32, name="srow_sb", tag="srow_sb", bufs=1)
            nc.vector.tensor_scalar_add(s0[:], srow[:], 1e-16)
            r0 = rowp.tile([1, C], FP32, name="rrow_sb", tag="rrow_sb", bufs=1)
            nc.vector.reciprocal(r0[:], s0[:])
            cols = colp.tile([128, TL], FP32, name="rcols", tag="rcols", bufs=2)
            for j in range(TL):
                # [1,128] row -> [128,1] col reshape via small SBUF->SBUF DMA
                nc.sync.dma_start(
                    cols[:, j : j + 1], r0[0:1, j * 128 : (j + 1) * 128]
                )
            return cols

        def attn_cols_from_hT(hT_sb, avec_sb, Kt):
            cols = colp.tile([128, TL], FP32, name="acols", tag="acols", bufs=4)
            for mn in range(TL):
                psc = pwork.tile([128, 1], FP32, name="acps", tag="wk", bufs=2)
                for kf in range(Kt):
                    MM(
                        psc[:],
                        hT_sb[:, kf * NL + mn * 128 : kf * NL + (mn + 1) * 128],
                        avec_sb[:, kf : kf + 1],
                        kf == 0,
                        kf == Kt - 1,
                    )
                nc.vector.tensor_copy(cols[:, mn : mn + 1], psc[:])
            return cols

        def attn_cols_dve(h_node_sb, ab_sb):
            cols = colp.tile([128, TL], FP32, name="dcols", tag="acols", bufs=4)
            for mn in range(TL):
                junk = sp.tile([128, C], FP32, name="ttr_junk", tag="junk", bufs=2)
                nc.vector.tensor_mul(junk[:], h_node_sb[:, mn * C : (mn + 1) * C], ab_sb[:])
                nc.vector.reduce_sum(cols[:, mn : mn + 1], junk[:], axis=mybir.AxisListType.X)
            return cols

        def cols_to_bcast(cols):
            row = rowp.tile([1, C], FP32, name="adrow", tag="adrow", bufs=1)
            for j in range(TL):
                pt = pwork.tile([1, 128], FP32, name="trrow", tag="wk", bufs=2)
                nc.tensor.transpose(pt[:], cols[:, j : j + 1], ident_f[:])
                nc.vector.tensor_copy(row[0:1, j * 128 : (j + 1) * 128], pt[:])
            pb = pwork.tile([128, C], FP32, name="adbc", tag="wk", bufs=2)
            MM(pb[:], ones_row_f[:], row[:], True, True)
            out = sp.tile([128, C], FP32, name="adstb", tag="adstb", bufs=1)
            nc.vector.tensor_copy(out[:], pb[:])
            return out

        def row_to_bcast_bf(row_sb):
            pb = pwork.tile([128, C], FP32, name="abps", tag="wk", bufs=2)
            MM(pb[:], ones_row_f[:], row_sb[:], True, True)
            out = sp.tile([128, C], BF, name="ab", tag="ab", bufs=2)
            nc.vector.tensor_copy(out[:], pb[:])
            return out

        def asrc_allgather(cols, name):
            bi = dramp.tile([NL, 1], FP32, name=f"agi_{name}", tag=f"agi_{name}")
            bo = dramp.tile(
                [N, 1], FP32, name=f"ago_{name}", tag=f"ago_{name}", addr_space="Shared"
            )
            for t in range(TL):
                nc.sync.dma_start(bi[t * 128 : (t + 1) * 128, :], cols[:, t : t + 1])
            nc.gpsimd.collective_compute(
                "AllGather", OP.bypass, replica_groups=RG, ins=[bi.opt()], outs=[bo.opt()]
            )
            asf = colp.tile([128, TN], FP32, name=f"asf_{name}", tag="asf", bufs=2)
            nc.sync.dma_start(
                asf[:].rearrange("p (t c) -> p t c", c=1),
                bo.rearrange("(t p) c -> p t c", p=128),
            )
            return asf

        def dma_tap(name, sb):
            if debug and name in taps:
                for t in range(TL):
                    nc.sync.dma_start(
                        taps[name].ap()[t * 128 : (t + 1) * 128, :], sb[:, t * C : (t + 1) * C]
                    )

        # ==================== L1: GCN1 (1024 -> 512), ELU ====================
        with nc.named_scope("L1_gcn1"):
            xT_sb = actp.tile([128, (D1 // 128) * NL], BF, name="xT_sb", tag="hT_in", bufs=2)
            dma_in_tiled(xT_sb, xT_d, D1, NL)
            w1 = load_w("w1_sb", w1_d, D1, D2)
            xw1T = actp.tile([128, TL * C], BF, name="xw1T", tag="pT", bufs=1)
            wmm(
                w1,
                D2,
                D1 // 128,
                xT_sb,
                lambda mt, ps: nc.vector.tensor_copy(xw1T[:, mt * NL : (mt + 1) * NL], ps[:]),
                rC=NL,
            )
            xw1 = actp.tile([128, TL * C], BF, name="xw1", tag="hN", bufs=2)
            transpose_blocks(xw1, C, xw1T, NL, TL, TL, BF)
            bo1 = allgather(xw1, NL, C, BF, "xw1")
            xw1f = load_full(bo1, "xw1full")

            h1 = actp.tile([128, TL * C], BF, name="h1", tag="hN", bufs=2)
            accs, _ = agg_run(stream_producer(agcnT_d), xw1f, True, False)
            for mn in range(TL):
                elu_from(h1[:, mn * C : (mn + 1) * C], accs[mn][:])
            dma_tap("tap_h1", h1)

        if nlayers < 2:
            return

        # ==================== L2: GCN2 (512 -> 1024) ====================
        with nc.named_scope("L2_gcn2"):
            bo2 = allgather(h1, NL, C, BF, "h1")
            h1f = load_full(bo2, "h1full")
            agT = actp.tile([128, TL * C], BF, name="agT", tag="pT", bufs=1)
            accs, _ = agg_run(stream_producer(agcnT_d), h1f, False, False)
            for mf in range(TL):
                nc.vector.tensor_copy(agT[:, mf * C : (mf + 1) * C], accs[mf][:])
            w2 = load_w("w2_sb", w2_d, D2, D1)
            h2T_b = actp.tile([128, (D1 // 128) * NL], BF, name="h2T_b", tag="hT_in", bufs=2)

            def ev2(mt, ps):
                out_evict(out_d["out2"], mt, ps, bf_dst=h2T_b)

            wmm(w2, D1, D2 // 128, agT, ev2, rC=NL)

        if nlayers < 3:
            return

        # ==================== L3: TransformerConv (1024 -> 512), ELU ====================
        with nc.named_scope("L3_tr3"):
            wq3 = load_w("wq3_sb", wq3_d, D1, D2)
            wk3 = load_w("wk3_sb", wk3_d, D1, D2)
            wv3 = load_w("wv3_sb", wv3_d, D1, D2)
            ws3 = load_w("ws3_sb", ws3_d, D1, D2)

            qT = actp.tile([128, TL * C], BF, name="qT", tag="qT", bufs=1)
            wmm(wq3, D2, D1 // 128, h2T_b, lambda mt, ps: nc.vector.tensor_copy(qT[:, mt * NL : (mt + 1) * NL], ps[:]), rC=NL)
            kT = actp.tile([128, TL * C], BF, name="kT", tag="kT", bufs=1)
            wmm(wk3, D2, D1 // 128, h2T_b, lambda mt, ps: nc.vector.tensor_copy(kT[:, mt * NL : (mt + 1) * NL], ps[:]), rC=NL)
            vT = actp.tile([128, TL * C], BF, name="vT", tag="pT", bufs=1)
            wmm(wv3, D2, D1 // 128, h2T_b, lambda mt, ps: nc.vector.tensor_copy(vT[:, mt * NL : (mt + 1) * NL], ps[:]), rC=NL)
            skT = actp.tile([128, TL * C], FP32, name="skT", tag="skT", bufs=1)
            wmm(ws3, D2, D1 // 128, h2T_b, lambda mt, ps: nc.vector.tensor_copy(skT[:, mt * NL : (mt + 1) * NL], ps[:]), rC=NL)

            bo_k = allgather(kT, D2, NL, BF, "kT3")
            kTf = load_full(bo_k, "kTfull3")
            v3 = actp.tile([128, TL * C], BF, name="v3", tag="hN", bufs=2)
            transpose_blocks(v3, C, vT, NL, TL, TL, BF)
            bo_v = allgather(v3, NL, C, BF, "v3")
            vf = load_full(bo_v, "vfull3")

            accs, srow = agg_run(
                tr_producer(mnoslT_d, kTf, qT, float(1.0 / np.sqrt(D2))), vf, False, True
            )
            rb = srow_to_rb(srow)
            h3T = actp.tile([128, TL * C], BF, name="h3T", tag="hT_in2", bufs=1)
            for mf in range(TL):
                t0 = sp.tile([128, C], FP32, name="t0_3", tag="t0", bufs=2)
                nc.vector.tensor_mul(t0[:], accs[mf][:], rb[:])
                nc.vector.tensor_add(t0[:], t0[:], skT[:, mf * NL : (mf + 1) * NL])
                elu_from(h3T[:, mf * NL : (mf + 1) * NL], t0[:])
            h3 = actp.tile([128, TL * C], BF, name="h3", tag="hN", bufs=2)
            transpose_blocks(h3, C, h3T, NL, TL, TL, BF)
            dma_tap("tap_h3", h3)

        if nlayers < 4:
            return

        # ==================== L4: TransformerConv (512 -> 1024) ====================
        with nc.named_scope("L4_tr4"):
            bo_h3 = allgather(h3, NL, C, BF, "h3")
            blin4 = load_w("blin4_sb", blin4_d, D2, D2)
            GT = actp.tile([128, TL * C], BF, name="GT", tag="pT", bufs=1)
            wmm(blin4, D2, D2 // 128, h3T, lambda mt, ps: nc.vector.tensor_copy(GT[:, mt * NL : (mt + 1) * NL], ps[:]), rC=NL)
            bo_g = allgather(GT, D2, NL, BF, "GT4")
            GTf = load_full(bo_g, "GTfull4")
            h3f = load_full(bo_h3, "h3full")

            accs, srow = agg_run(
                tr_producer(mnoslT_d, GTf, h3T, float(1.0 / np.sqrt(D1))), h3f, False, True
            )
            rb = srow_to_rb(srow)
            agTn = actp.tile([128, TL * C], BF, name="agTn", tag="agT", bufs=1)
            for mf in range(TL):
                nc.vector.tensor_mul(agTn[:, mf * C : (mf + 1) * C], accs[mf][:], rb[:])

            wv4 = load_w("wv4_sb", wv4_d, D2, D1)
            ws4 = load_w("ws4_sb", ws4_d, D2, D1)
            h4T_b = actp.tile([128, (D1 // 128) * NL], BF, name="h4T_b", tag="hT_in", bufs=2)
            for mt in range(D1 // 128):
                ps = pwork.tile([128, NL], FP32, name="mm4", tag="wk", bufs=2)
                for kt in range(D2 // 128):
                    MM(
                        ps[:],
                        wv4[:, kt * D1 + mt * 128 : kt * D1 + (mt + 1) * 128],
                        agTn[:, kt * C : (kt + 1) * C],
                        kt == 0,
                        False,
                    )
                for kt in range(D2 // 128):
                    MM(
                        ps[:],
                        ws4[:, kt * D1 + mt * 128 : kt * D1 + (mt + 1) * 128],
                        h3T[:, kt * NL : (kt + 1) * NL],
                        False,
                        kt == D2 // 128 - 1,
                    )
                out_evict(out_d["out4"], mt, ps, bf_dst=h4T_b)

        if nlayers < 5:
            return

        # ==================== GAT layers ====================
        def gat_project_first(lname, hT_in, w_d_, as_d, ad_d, mask_d, tapname):
            w_sb = load_w(f"w_{lname}", w_d_, D1, D2)
            hpT = actp.tile([128, TL * C], BF, name=f"hpT_{lname}", tag="pT", bufs=1)
            wmm(
                w_sb,
                D2,
                D1 // 128,
                hT_in,
                lambda mt, ps: nc.vector.tensor_copy(hpT[:, mt * NL : (mt + 1) * NL], ps[:]),
                rC=NL,
            )
            avs = colp.tile([128, TL], BF, name=f"avs_{lname}", tag="av", bufs=4)
            nc.sync.dma_start(
                avs[:].rearrange("p (k c) -> p k c", c=1),
                as_d.ap().rearrange("(k p) c -> p k c", p=128),
            )
            avd = colp.tile([128, TL], BF, name=f"avd_{lname}", tag="av", bufs=4)
            nc.sync.dma_start(
                avd[:].rearrange("p (k c) -> p k c", c=1),
                ad_d.ap().rearrange("(k p) c -> p k c", p=128),
            )
            asrc_cols = attn_cols_from_hT(hpT, avs, TL)
            adst_cols = attn_cols_from_hT(hpT, avd, TL)
            asf = asrc_allgather(asrc_cols, f"as_{lname}")
            adstb = cols_to_bcast(adst_cols)

            hp = actp.tile([128, TL * C], BF, name=f"hp_{lname}", tag="hN", bufs=2)
            transpose_blocks(hp, C, hpT, NL, TL, TL, BF)
            bo = allgather(hp, NL, C, BF, f"hp_{lname}")
            hpf = load_full(bo, f"hpf_{lname}")

            accs, srow = agg_run(gat_producer(mask_d, adstb, asf), hpf, True, True)
            rcols = srow_to_cols(srow)
            h_out = actp.tile([128, TL * C], BF, name=f"hout_{lname}", tag="hN", bufs=2)
            for mn in range(TL):
                t0 = sp.tile([128, C], FP32, name=f"t0_{lname}", tag="t0", bufs=2)
                nc.vector.tensor_scalar_mul(t0[:], accs[mn][:], rcols[:, mn : mn + 1])
                elu_from(h_out[:, mn * C : (mn + 1) * C], t0[:])
            dma_tap(tapname, h_out)
            return h_out

        def gat_agg_first(lname, h_node, as_row_d, ad_row_d, w_d_, mask_d, out_name, next_bf):
            r_s = rowp.tile([1, C], FP32, name=f"ars_{lname}", tag="ar_row", bufs=2)
            nc.sync.dma_start(r_s[:], as_row_d.ap())
            r_d = rowp.tile([1, C], FP32, name=f"ard_{lname}", tag="ar_row", bufs=2)
            nc.sync.dma_start(r_d[:], ad_row_d.ap())
            ab_s = row_to_bcast_bf(r_s)
            ab_d = row_to_bcast_bf(r_d)

            asrc_cols = attn_cols_dve(h_node, ab_s)
            adst_cols = attn_cols_dve(h_node, ab_d)
            asf = asrc_allgather(asrc_cols, f"as_{lname}")
            adstb = cols_to_bcast(adst_cols)

            bo = allgather(h_node, NL, C, BF, f"h_{lname}")
            hf = load_full(bo, f"hf_{lname}")

            accs, srow = agg_run(gat_producer(mask_d, adstb, asf), hf, False, True)
            rb = srow_to_rb(srow)
            agTn_ = actp.tile([128, TL * C], BF, name=f"agTn_{lname}", tag="agT", bufs=1)
            for mf in range(TL):
                nc.vector.tensor_mul(agTn_[:, mf * C : (mf + 1) * C], accs[mf][:], rb[:])

            w_sb = load_w(f"w_{lname}", w_d_, D2, D1)
            hT_b = None
            if next_bf:
                hT_b = actp.tile([128, (D1 // 128) * NL], BF, name=f"hTb_{lname}", tag="hT_in", bufs=2)

            def evw(mt, ps):
                out_evict(out_d[out_name], mt, ps, bf_dst=hT_b)

            wmm(w_sb, D1, D2 // 128, agTn_, evw, rC=NL)
            return hT_b

        with nc.named_scope("L5_gat5"):
            h5 = gat_project_first("g5", h4T_b, w5_d, a5s_d, a5d_d, mslT_d, "tap_h5")
        if nlayers < 6:
            return
        with nc.named_scope("L6_gat6"):
            h6T_b = gat_agg_first("g6", h5, a6s_d, a6d_d, w6_d, mslT_d, "out6", next_bf=True)
        if nlayers < 7:
            return
        with nc.named_scope("L7_gat7"):
            h7 = gat_project_first("g7", h6T_b, w7_d, a7s_d, a7d_d, mslT_d, "tap_h7")
        if nlayers < 8:
            return
        with nc.named_scope("L8_gat8"):
            gat_agg_first("g8", h7, a8s_d, a8d_d, w8_d, mslT_d, "out8", next_bf=False)

    with tile.TileContext(nc) as tc, ExitStack() as ctx:
        _body(tc, ctx)

    nc.compile()
    nc.m = get_hw_module(nc.m)
    return nc


# ----------------------------------------------------------------------------
# Runner
# ----------------------------------------------------------------------------


def _install_profile_shim():
    import types
    from trn_agent_boot.trn_boot import _ntff_profile_via_ctypes

    if "antenv.axon_hooks" in sys.modules:
        return
    try:
        hook = _ntff_profile_via_ctypes("/opt/axon/libaxon_pjrt.so")
    except OSError:
        hook = None
    mod = types.ModuleType("antenv.axon_hooks")
    mod.get_axon_ntff_profile_hook = lambda: hook
    sys.modules["antenv.axon_hooks"] = mod
    bass_utils.upload_artifacts = lambda tmpdir: "local://" + tmpdir


def _get_program(nlayers=8, debug=False):
    key = (nlayers, debug)
    if key not in _CACHE:
        _CACHE[key] = _build(nlayers=nlayers, debug=debug)
    return _CACHE[key]


def _run(inputs, trace=False, nlayers=8, debug=False):
    nc = _get_program(nlayers=nlayers, debug=debug)
    in_maps = _host_prep(inputs["x"], inputs["edge_index"], inputs["params"])
    if trace:
        _install_profile_shim()
    res = bass_utils.run_bass_kernel_spmd(
        nc, in_maps, core_ids=list(range(NCORES)), trace=trace
    )
    return res


def kernel(x, edge_index, params):
    res = _run({"x": x, "edge_index": edge_index, "params": params}, trace=False)
    outs = []
    for name in ("out2", "out4", "out6", "out8"):
        full = np.concatenate(
            [np.asarray(res.results[c][name]).T for c in range(NCORES)], axis=0
        )
        outs.append(full[None].astype(np.float32))
    return tuple(outs)
